# revision 20
# baseline (speedup 1.0000x reference)
"""Trainium2 Bass kernel for the Neural-CDE-style cell (nn_JaCDE_88167088653055).

Math (per batch row b):
    x    = spline(coeffs, t)   xdot = spline(dcoeffs, t)
    l1   = x @ wx.T + h @ wh.T + b0
    relu = relu(l1);  dr = sigmoid(l1)
    lout = relu @ wout.T + b1; dth = 1 - tanh(lout)^2
    m1   = (dr*(xdot @ wx.T)) @ wout.T
    jx   = dth*m1;  m2 = (dr*(jx @ wh.T)) @ wout.T
    jxh  = dth*m2;  m3 = (dr*(jxh @ wh.T)) @ wout.T
    out  = dth*(m1 + m2 + m3)

Device-side reformulation (bf16 PE path; tolerance 2e-2):
  * host does the spline; x/xdot stack on partitions 0:64 / 64:128.
  * single ACT table set: walrus maps funcs first-fit to table sets
    (Relu/Tanh/Square -> exp_and_others, Sigmoid -> sigmoid_and_others), so
    sigmoid is computed as dr = (1+tanh(l1/2))/2: every dr-multiply is an
    STT (tt+1)*g = 2*dr*g and the extra 2 folds into the wout copies
    (won2 = -wout.T/2).  dth-multiplies are STT (sq-1)*(-m) with
    sq = Square(tanh(lout)).  -> ONE ACT_TABLE_LOAD, during the DMA wait.
  * relu runs on the (otherwise idle pre-chain) DVE as STT (l1+b0) max 0,
    cutting the serial ACT queue from 8 ops to 6.
  * m1+m2+m3 accumulates IN PSUM: A = m1; (read jx); A += m2; (read
    jx+jxh -> feeds g1+g2 so the next matmul produces m2+m3); A += wop*p2
    (subtracts the extra m2) and A += won*p23. No vector adds at all.
  * all matmuls are single N=512; batch = 2 chains of 512 per core.
  * small HAM warm-up trickle fills the input-DMA dead window.
  * one input DMA per pair (pair0 sync ring, pair1 gpsimd SWDGE);
    weights on the scalar ring; outputs DMA per pair as soon as ready.

Sharding: pure data parallel - batch 8192 = 8 cores x 1024 rows; weights
replicated. Activations feature-major ([H or CIN on partitions, batch free]);
every matmul is out.T = W @ act.T with contraction on partitions.
"""

import ml_dtypes
import numpy as np

import concourse.bass as bass
import concourse.mybir as mybir
import concourse.tile as tile
from concourse import bacc, bass_utils

N_CORES = 8
B = 8192
NOBS = 16
CIN = 64
H = 128
BS = B // N_CORES       # 1024 batch rows per core
PAIR = 512              # batch columns per chain (= one PSUM bank)
NPAIR = BS // PAIR      # 2 chains per core
PACKW = 2 * PAIR        # input pack: [x|xd (512) | h (512)]
F32 = mybir.dt.float32
BF16 = mybir.dt.bfloat16
NPBF = ml_dtypes.bfloat16

N_WARM = 14             # HAM warm-up trickle fills the input-DMA dead window

_NC_CACHE = {}


def _build_nc():
    AF = mybir.ActivationFunctionType
    OP = mybir.AluOpType

    nc = bacc.Bacc("TRN2", target_bir_lowering=False, debug=False,
                   enable_asserts=False, num_devices=N_CORES)

    # per-pair input pack: cols 0:512 = x.T (parts 0:64) | xdot.T (64:128),
    # cols 512:1024 = h.T
    inb = nc.dram_tensor("inb", [NPAIR, 128, PACKW], BF16,
                         kind="ExternalInput")
    # [wxx | whT | won2 | wop2 | woT]; wxx rows 0:64 = wx.T (x), 64:128 =
    # wx.T (xdot); won2 = -0.5*wout.T, wop2 = +0.5*wout.T, woT = wout.T
    wpack = nc.dram_tensor("wpack", [128, 5 * H], BF16, kind="ExternalInput")
    bpack = nc.dram_tensor("bpack", [128, 4], F32, kind="ExternalInput")
    outt = nc.dram_tensor("outt", [H, BS], BF16, kind="ExternalOutput")

    def mm(out_ap, lhsT, rhs, start=True, stop=True):
        nc.tensor.matmul(out_ap, lhsT, rhs, start=start, stop=stop,
                         skip_group_check=True)

    with tile.TileContext(nc) as tc:
        with tc.tile_pool(name="w", bufs=1) as wp, \
             tc.tile_pool(name="io", bufs=2) as io, \
             tc.tile_pool(name="tmp", bufs=2) as tmp, \
             tc.tile_pool(name="fr", bufs=4, space="PSUM") as fr, \
             tc.tile_pool(name="acc", bufs=2, space="PSUM") as accp, \
             tc.tile_pool(name="g", bufs=2, space="PSUM") as gp:

            # --- weights / biases on the scalar (ACT) HWDGE ring ----------
            ws = wp.tile([128, 5 * H], BF16, tag="ws")
            nc.scalar.dma_start(ws[:], wpack[:])
            bs_ = wp.tile([128, 4], F32, tag="bs")
            wxx_x = ws[0:64, 0:H]
            wxx_xd = ws[64:128, 0:H]
            whT = ws[:, H:2 * H]
            won2 = ws[:, 2 * H:3 * H]
            wop2 = ws[:, 3 * H:4 * H]
            woT = ws[:, 4 * H:5 * H]
            b0s = bs_[:, 0:1]
            b0h = bs_[:, 1:2]    # = b0/2
            b1s = bs_[:, 2:3]    # = b1
            mone = bs_[:, 3:4]   # = -1.0

            # --- input DMAs ----------------------------------------------
            # pair0 is split across BOTH HWDGE rings so its two halves
            # transfer in parallel; pair1 rides gpsimd but only after a
            # memset of its own tile (WAW-serialized) so it does not steal
            # HBM bandwidth from pair0's critical-path transfer.
            it = [io.tile([128, PACKW], BF16, tag="it", name=f"it{p}")
                  for p in range(NPAIR)]
            nc.sync.dma_start(it[0][:, 0:PAIR], inb[0, :, 0:PAIR])
            nc.scalar.dma_start(it[0][:, PAIR:PACKW], inb[0, :, PAIR:PACKW])
            nc.gpsimd.memset(it[1][:], 0.0)
            nc.gpsimd.dma_start(it[1][:], inb[1])
            nc.scalar.dma_start(bs_[:], bpack[:])

            # memset scratch (DVE is idle): trickle weights + relu zeros
            wdum = wp.tile([128, 128], BF16, tag="wdum")
            nc.vector.memset(wdum[:], 0.0)
            zs = wp.tile([128, PAIR], BF16, tag="zs")
            nc.vector.memset(zs[:], 0.0)
            # --- HAM warm-up trickle (PE queue head) ----------------------
            dumg = gp.tile([H, PAIR], F32, tag="g")
            for _ in range(N_WARM):
                mm(dumg[:, 0:128], wdum[:], wdum[:], start=True, stop=True)

            T = {}

            def tt(name, p, dtype=BF16):
                t = tmp.tile([H, PAIR], dtype, tag=name, name=f"{name}{p}")
                T[(name, p)] = t
                return t

            pairs = range(NPAIR)

            # --- front matmuls --------------------------------------------
            def front_mms(p):
                L = fr.tile([H, PAIR], F32, tag="fr", name=f"L{p}")
                T[("L", p)] = L
                mm(L[:], wxx_x, it[p][0:64, 0:PAIR], start=True, stop=False)
                mm(L[:], whT, it[p][:, PAIR:PACKW], start=False, stop=True)
                # U shares the acc pool: A[p] reuses U[p]'s bank, and the
                # WAR (A's first matmul after p1's read of U) is a true
                # data dependency anyway.  U (K=64, row group 2/3) runs
                # concurrently with a K=64 row-group-0/1 matmul.
                U = accp.tile([H, PAIR], F32, tag="acc", name=f"U{p}")
                T[("U", p)] = U
                mm(U[:], wxx_xd, it[p][64:128, 0:PAIR], start=True, stop=True)

            # --- relu on DVE: (l1 + b0) max 0 -----------------------------
            def relu(p):
                nc.vector.scalar_tensor_tensor(
                    tt("relu", p)[:], T[("L", p)][:], b0s, zs[:],
                    OP.add, OP.max)

            # --- ACT front: tt/th/sq only (all exp_and_others) ------------
            def act_tt(p):
                nc.scalar.activation(tt("tt", p)[:], T[("L", p)][:],
                                     AF.Tanh, bias=b0h, scale=0.5)

            def lo_mm(p):
                LO = fr.tile([H, PAIR], F32, tag="fr", name=f"LO{p}")
                T[("LO", p)] = LO
                mm(LO[:], woT, T[("relu", p)][:], start=True, stop=True)

            def act_th_sq(p):
                nc.scalar.activation(tt("th", p)[:], T[("LO", p)][:],
                                     AF.Tanh, bias=b1s)
                nc.scalar.activation(tt("sq", p)[:], T[("th", p)][:],
                                     AF.Square, bias=0.0)

            A = {}

            def p1(p):
                # (tt+1)*u = 2*dr*u; the 2 folds into won2/wop2
                nc.vector.scalar_tensor_tensor(
                    tt("p1", p)[:], T[("tt", p)][:], 1.0, T[("U", p)][:],
                    OP.add, OP.mult)

            def y(name, p):
                # (sq-1)*A = (1-tanh^2)*m with A = -(m1+...)
                nc.vector.scalar_tensor_tensor(
                    tt(name, p)[:], T[("sq", p)][:], 1.0, A[p][:],
                    OP.subtract, OP.mult)

            def pg(name, p, gt):
                nc.vector.scalar_tensor_tensor(
                    tt(name, p)[:], T[("tt", p)][:], 1.0, gt[:],
                    OP.add, OP.mult)

            # front, in expected-readiness order; pair0's LO matmul sits
            # ahead of pair1's front matmuls in the PE FIFO so pair0's
            # th/sq path is not blocked behind them.
            front_mms(0)
            relu(0)                                   # DVE
            act_tt(0)                                 # ACT
            lo_mm(0)                                  # PE
            act_th_sq(0)                              # ACT
            front_mms(1)
            relu(1)                                   # DVE
            act_tt(1)                                 # ACT
            lo_mm(1)                                  # PE
            act_th_sq(1)                              # ACT

            for p in pairs:
                A[p] = accp.tile([H, PAIR], F32, tag="acc", name=f"A{p}")
            G = {}

            def g_mm(name, p, src):
                G[(name, p)] = gp.tile([H, PAIR], F32, tag="g",
                                       name=f"{name}_{p}")
                mm(G[(name, p)][:], whT, src[:], start=True, stop=True)

            # --- Jacobian chains (A accumulates -(m1+m2+m3)) --------------
            p1(0)                                                  # DVE
            mm(A[0][:], won2, T[("p1", 0)][:], start=True, stop=False)
            y("y1", 0)                                             # DVE
            g_mm("G1", 0, T[("y1", 0)])
            p1(1)                                                  # DVE
            mm(A[1][:], won2, T[("p1", 1)][:], start=True, stop=False)
            pg("p2", 0, G[("G1", 0)])                              # DVE
            mm(A[0][:], won2, T[("p2", 0)][:], start=False, stop=False)
            y("y1", 1)                                             # DVE
            g_mm("G1", 1, T[("y1", 1)])
            y("y2", 0)                                             # DVE
            g_mm("G2", 0, T[("y2", 0)])
            mm(A[0][:], wop2, T[("p2", 0)][:], start=False, stop=False)
            pg("p2", 1, G[("G1", 1)])                              # DVE
            mm(A[1][:], won2, T[("p2", 1)][:], start=False, stop=False)
            pg("p23", 0, G[("G2", 0)])                             # DVE
            mm(A[0][:], won2, T[("p23", 0)][:], start=False, stop=True)
            y("y2", 1)                                             # DVE
            g_mm("G2", 1, T[("y2", 1)])
            mm(A[1][:], wop2, T[("p2", 1)][:], start=False, stop=False)
            y("out", 0)                                            # DVE
            nc.sync.dma_start(outt[:, 0:PAIR], T[("out", 0)][:])
            pg("p23", 1, G[("G2", 1)])                             # DVE
            mm(A[1][:], won2, T[("p23", 1)][:], start=False, stop=True)
            y("out", 1)                                            # DVE
            nc.scalar.dma_start(outt[:, PAIR:2 * PAIR], T[("out", 1)][:])

    nc.compile()
    return nc


def _get_nc():
    if "nc" not in _NC_CACHE:
        _NC_CACHE["nc"] = _build_nc()
    return _NC_CACHE["nc"]


def _prep_in_maps(t, h, coeffs, dcoeffs, tobs, wx, wh, wout, b0, b1):
    t = np.asarray(t, np.float32)
    h = np.asarray(h, np.float32)
    coeffs = np.asarray(coeffs, np.float32)
    dcoeffs = np.asarray(dcoeffs, np.float32)
    tobs = np.asarray(tobs, np.float32)
    wx = np.asarray(wx, np.float32)
    wh = np.asarray(wh, np.float32)
    wout = np.asarray(wout, np.float32)
    b0 = np.asarray(b0, np.float32)
    b1 = np.asarray(b1, np.float32)

    ts = t[0]
    idx = int(np.clip(np.searchsorted(tobs, ts, side="right") - 1, 0, NOBS - 2))
    dtv = np.float32(ts - tobs[idx])
    powers = dtv ** np.arange(4, dtype=np.float32)            # [4]

    # host-side spline eval
    x = coeffs[:, idx] @ powers                               # [B, CIN]
    xdot = dcoeffs[:, idx] @ powers                           # [B, CIN]

    wxx = np.concatenate([wx.T, wx.T], axis=0)                # [128, 128]
    wpack = np.concatenate(
        [wxx, wh.T, -0.5 * wout.T, 0.5 * wout.T, wout.T],
        axis=1).astype(NPBF)                                  # [128, 640]
    bpack = np.stack([b0, 0.5 * b0, b1, np.full(H, -1.0, np.float32)],
                     axis=1).astype(np.float32)

    xb = x.astype(NPBF)
    xdb = xdot.astype(NPBF)
    hb = h.astype(NPBF)

    in_maps = []
    for c in range(N_CORES):
        sl = slice(c * BS, (c + 1) * BS)
        xt = xb[sl].T                                         # [64, BS]
        xdt = xdb[sl].T
        ht = hb[sl].T                                         # [128, BS]
        inb = np.empty((NPAIR, 128, PACKW), NPBF)
        for p in range(NPAIR):
            cls = slice(p * PAIR, (p + 1) * PAIR)
            inb[p, 0:64, 0:PAIR] = xt[:, cls]
            inb[p, 64:128, 0:PAIR] = xdt[:, cls]
            inb[p, :, PAIR:PACKW] = ht[:, cls]
        in_maps.append({"inb": inb, "wpack": wpack, "bpack": bpack})
    return in_maps


def kernel(**inputs) -> np.ndarray:
    in_maps = _prep_in_maps(**inputs)
    nc = _get_nc()
    res = bass_utils.run_bass_kernel_spmd(nc, in_maps,
                                          core_ids=list(range(N_CORES)))
    out = np.empty((B, H), np.float32)
    for c in range(N_CORES):
        out[c * BS:(c + 1) * BS] = res.results[c]["outt"].T.astype(np.float32)
    return out


# revision 21
# speedup vs baseline: 1.0447x; 1.0447x over previous
"""Trainium2 Bass kernel for the Neural-CDE-style cell (nn_JaCDE_88167088653055).

Math (per batch row b):
    x    = spline(coeffs, t)   xdot = spline(dcoeffs, t)
    l1   = x @ wx.T + h @ wh.T + b0
    relu = relu(l1);  dr = sigmoid(l1)
    lout = relu @ wout.T + b1; dth = 1 - tanh(lout)^2
    m1   = (dr*(xdot @ wx.T)) @ wout.T
    jx   = dth*m1;  m2 = (dr*(jx @ wh.T)) @ wout.T
    jxh  = dth*m2;  m3 = (dr*(jxh @ wh.T)) @ wout.T
    out  = dth*(m1 + m2 + m3)

Device-side reformulation (bf16 PE path; tolerance 2e-2):
  * host does the spline; x/xdot stack on partitions 0:64 / 64:128.
  * single ACT table set: walrus maps funcs first-fit to table sets
    (Relu/Tanh/Square -> exp_and_others, Sigmoid -> sigmoid_and_others), so
    sigmoid is computed as dr = (1+tanh(l1/2))/2: every dr-multiply is an
    STT (tt+1)*g = 2*dr*g and the extra 2 folds into the wout copies
    (won2 = -wout.T/2).  dth-multiplies are STT (sq-1)*(-m) with
    sq = Square(tanh(lout)).  -> ONE ACT_TABLE_LOAD, during the DMA wait.
  * relu runs on the (otherwise idle pre-chain) DVE as STT (l1+b0) max 0,
    cutting the serial ACT queue from 8 ops to 6.
  * m1+m2+m3 accumulates IN PSUM: A = m1; (read jx); A += m2; (read
    jx+jxh -> feeds g1+g2 so the next matmul produces m2+m3); A += wop*p2
    (subtracts the extra m2) and A += won*p23. No vector adds at all.
  * all matmuls are single N=512; batch = 2 chains of 512 per core.
  * small HAM warm-up trickle fills the input-DMA dead window.
  * one input DMA per pair (pair0 sync ring, pair1 gpsimd SWDGE);
    weights on the scalar ring; outputs DMA per pair as soon as ready.

Sharding: pure data parallel - batch 8192 = 8 cores x 1024 rows; weights
replicated. Activations feature-major ([H or CIN on partitions, batch free]);
every matmul is out.T = W @ act.T with contraction on partitions.
"""

import ml_dtypes
import numpy as np

import concourse.bass as bass
import concourse.mybir as mybir
import concourse.tile as tile
from concourse import bacc, bass_utils

N_CORES = 8
B = 8192
NOBS = 16
CIN = 64
H = 128
BS = B // N_CORES       # 1024 batch rows per core
PAIR = 512              # batch columns per chain (= one PSUM bank)
NPAIR = BS // PAIR      # 2 chains per core
PACKW = 2 * PAIR        # input pack: [x|xd (512) | h (512)]
F32 = mybir.dt.float32
BF16 = mybir.dt.bfloat16
NPBF = ml_dtypes.bfloat16

N_WARM = 16             # HAM warm-up trickle fills the input-DMA dead window

_NC_CACHE = {}


def _build_nc():
    AF = mybir.ActivationFunctionType
    OP = mybir.AluOpType

    nc = bacc.Bacc("TRN2", target_bir_lowering=False, debug=False,
                   enable_asserts=False, num_devices=N_CORES)

    # per-pair input pack: cols 0:512 = x.T (parts 0:64) | xdot.T (64:128),
    # cols 512:1024 = h.T
    inb = nc.dram_tensor("inb", [NPAIR, 128, PACKW], BF16,
                         kind="ExternalInput")
    # [wxx | whT | won2 | wop2 | woT]; wxx rows 0:64 = wx.T (x), 64:128 =
    # wx.T (xdot); won2 = -0.5*wout.T, wop2 = +0.5*wout.T, woT = wout.T
    wpack = nc.dram_tensor("wpack", [128, 5 * H], BF16, kind="ExternalInput")
    bpack = nc.dram_tensor("bpack", [128, 4], F32, kind="ExternalInput")
    outt = nc.dram_tensor("outt", [H, BS], BF16, kind="ExternalOutput")

    def mm(out_ap, lhsT, rhs, start=True, stop=True):
        nc.tensor.matmul(out_ap, lhsT, rhs, start=start, stop=stop,
                         skip_group_check=True)

    with tile.TileContext(nc) as tc:
        with tc.tile_pool(name="w", bufs=1) as wp, \
             tc.tile_pool(name="io", bufs=2) as io, \
             tc.tile_pool(name="tmp", bufs=2) as tmp, \
             tc.tile_pool(name="fr", bufs=4, space="PSUM") as fr, \
             tc.tile_pool(name="acc", bufs=2, space="PSUM") as accp, \
             tc.tile_pool(name="g", bufs=2, space="PSUM") as gp:

            # --- weights / biases on the scalar (ACT) HWDGE ring ----------
            ws = wp.tile([128, 5 * H], BF16, tag="ws")
            nc.scalar.dma_start(ws[:], wpack[:])
            bs_ = wp.tile([128, 4], F32, tag="bs")
            nc.scalar.dma_start(bs_[:], bpack[:])
            wxx_x = ws[0:64, 0:H]
            wxx_xd = ws[64:128, 0:H]
            whT = ws[:, H:2 * H]
            won2 = ws[:, 2 * H:3 * H]
            wop2 = ws[:, 3 * H:4 * H]
            woT = ws[:, 4 * H:5 * H]
            b0s = bs_[:, 0:1]
            b0h = bs_[:, 1:2]    # = b0/2
            b1s = bs_[:, 2:3]    # = b1
            mone = bs_[:, 3:4]   # = -1.0

            # --- input DMAs: one per pair, separate rings -----------------
            it = [io.tile([128, PACKW], BF16, tag="it", name=f"it{p}")
                  for p in range(NPAIR)]
            nc.sync.dma_start(it[0][:], inb[0])
            nc.gpsimd.dma_start(it[1][:], inb[1])

            # memset scratch (DVE is idle): trickle weights + relu zeros
            wdum = wp.tile([128, 128], BF16, tag="wdum")
            nc.vector.memset(wdum[:], 0.0)
            zs = wp.tile([128, PAIR], BF16, tag="zs")
            nc.vector.memset(zs[:], 0.0)
            # --- HAM warm-up trickle (PE queue head) ----------------------
            dumg = gp.tile([H, PAIR], F32, tag="g")
            for _ in range(N_WARM):
                mm(dumg[:, 0:128], wdum[:], wdum[:], start=True, stop=True)

            T = {}

            def tt(name, p, dtype=BF16):
                t = tmp.tile([H, PAIR], dtype, tag=name, name=f"{name}{p}")
                T[(name, p)] = t
                return t

            pairs = range(NPAIR)

            # --- front matmuls --------------------------------------------
            def front_mms(p):
                L = fr.tile([H, PAIR], F32, tag="fr", name=f"L{p}")
                T[("L", p)] = L
                mm(L[:], wxx_x, it[p][0:64, 0:PAIR], start=True, stop=False)
                mm(L[:], whT, it[p][:, PAIR:PACKW], start=False, stop=True)
                # U shares the acc pool: A[p] reuses U[p]'s bank, and the
                # WAR (A's first matmul after p1's read of U) is a true
                # data dependency anyway.  U (K=64, row group 2/3) runs
                # concurrently with a K=64 row-group-0/1 matmul.
                U = accp.tile([H, PAIR], F32, tag="acc", name=f"U{p}")
                T[("U", p)] = U
                mm(U[:], wxx_xd, it[p][64:128, 0:PAIR], start=True, stop=True)

            # --- relu on DVE: (l1 + b0) max 0 -----------------------------
            def relu(p):
                nc.vector.scalar_tensor_tensor(
                    tt("relu", p)[:], T[("L", p)][:], b0s, zs[:],
                    OP.add, OP.max)

            # --- ACT front: tt/th/sq only (all exp_and_others) ------------
            def act_tt(p):
                nc.scalar.activation(tt("tt", p)[:], T[("L", p)][:],
                                     AF.Tanh, bias=b0h, scale=0.5)

            def lo_mm(p):
                LO = fr.tile([H, PAIR], F32, tag="fr", name=f"LO{p}")
                T[("LO", p)] = LO
                mm(LO[:], woT, T[("relu", p)][:], start=True, stop=True)

            def act_th_sq(p):
                nc.scalar.activation(tt("th", p)[:], T[("LO", p)][:],
                                     AF.Tanh, bias=b1s)
                nc.scalar.activation(tt("sq", p)[:], T[("th", p)][:],
                                     AF.Square, bias=0.0)

            A = {}

            def p1(p):
                # (tt+1)*u = 2*dr*u; the 2 folds into won2/wop2
                nc.vector.scalar_tensor_tensor(
                    tt("p1", p)[:], T[("tt", p)][:], 1.0, T[("U", p)][:],
                    OP.add, OP.mult)

            def y(name, p):
                # (sq-1)*A = (1-tanh^2)*m with A = -(m1+...)
                nc.vector.scalar_tensor_tensor(
                    tt(name, p)[:], T[("sq", p)][:], 1.0, A[p][:],
                    OP.subtract, OP.mult)

            def pg(name, p, gt):
                nc.vector.scalar_tensor_tensor(
                    tt(name, p)[:], T[("tt", p)][:], 1.0, gt[:],
                    OP.add, OP.mult)

            # front, in expected-readiness order; pair0's LO matmul sits
            # ahead of pair1's front matmuls in the PE FIFO so pair0's
            # th/sq path is not blocked behind them.
            front_mms(0)
            front_mms(1)
            relu(0)                                   # DVE
            act_tt(0)                                 # ACT
            lo_mm(0)                                  # PE
            act_tt(1)                                 # ACT
            relu(1)                                   # DVE
            act_th_sq(0)                              # ACT
            lo_mm(1)                                  # PE
            act_th_sq(1)                              # ACT

            for p in pairs:
                A[p] = accp.tile([H, PAIR], F32, tag="acc", name=f"A{p}")
            G = {}

            def g_mm(name, p, src):
                G[(name, p)] = gp.tile([H, PAIR], F32, tag="g",
                                       name=f"{name}_{p}")
                mm(G[(name, p)][:], whT, src[:], start=True, stop=True)

            # --- Jacobian chains (A accumulates -(m1+m2+m3)) --------------
            p1(0)                                                  # DVE
            mm(A[0][:], won2, T[("p1", 0)][:], start=True, stop=False)
            y("y1", 0)                                             # DVE
            g_mm("G1", 0, T[("y1", 0)])
            p1(1)                                                  # DVE
            mm(A[1][:], won2, T[("p1", 1)][:], start=True, stop=False)
            pg("p2", 0, G[("G1", 0)])                              # DVE
            mm(A[0][:], won2, T[("p2", 0)][:], start=False, stop=False)
            y("y1", 1)                                             # DVE
            g_mm("G1", 1, T[("y1", 1)])
            y("y2", 0)                                             # DVE
            g_mm("G2", 0, T[("y2", 0)])
            mm(A[0][:], wop2, T[("p2", 0)][:], start=False, stop=False)
            pg("p2", 1, G[("G1", 1)])                              # DVE
            mm(A[1][:], won2, T[("p2", 1)][:], start=False, stop=False)
            pg("p23", 0, G[("G2", 0)])                             # DVE
            mm(A[0][:], won2, T[("p23", 0)][:], start=False, stop=True)
            y("y2", 1)                                             # DVE
            g_mm("G2", 1, T[("y2", 1)])
            mm(A[1][:], wop2, T[("p2", 1)][:], start=False, stop=False)
            y("out", 0)                                            # DVE
            nc.sync.dma_start(outt[:, 0:PAIR], T[("out", 0)][:])
            pg("p23", 1, G[("G2", 1)])                             # DVE
            mm(A[1][:], won2, T[("p23", 1)][:], start=False, stop=True)
            y("out", 1)                                            # DVE
            nc.scalar.dma_start(outt[:, PAIR:2 * PAIR], T[("out", 1)][:])

    nc.compile()
    return nc


def _get_nc():
    if "nc" not in _NC_CACHE:
        _NC_CACHE["nc"] = _build_nc()
    return _NC_CACHE["nc"]


def _prep_in_maps(t, h, coeffs, dcoeffs, tobs, wx, wh, wout, b0, b1):
    t = np.asarray(t, np.float32)
    h = np.asarray(h, np.float32)
    coeffs = np.asarray(coeffs, np.float32)
    dcoeffs = np.asarray(dcoeffs, np.float32)
    tobs = np.asarray(tobs, np.float32)
    wx = np.asarray(wx, np.float32)
    wh = np.asarray(wh, np.float32)
    wout = np.asarray(wout, np.float32)
    b0 = np.asarray(b0, np.float32)
    b1 = np.asarray(b1, np.float32)

    ts = t[0]
    idx = int(np.clip(np.searchsorted(tobs, ts, side="right") - 1, 0, NOBS - 2))
    dtv = np.float32(ts - tobs[idx])
    powers = dtv ** np.arange(4, dtype=np.float32)            # [4]

    # host-side spline eval
    x = coeffs[:, idx] @ powers                               # [B, CIN]
    xdot = dcoeffs[:, idx] @ powers                           # [B, CIN]

    wxx = np.concatenate([wx.T, wx.T], axis=0)                # [128, 128]
    wpack = np.concatenate(
        [wxx, wh.T, -0.5 * wout.T, 0.5 * wout.T, wout.T],
        axis=1).astype(NPBF)                                  # [128, 640]
    bpack = np.stack([b0, 0.5 * b0, b1, np.full(H, -1.0, np.float32)],
                     axis=1).astype(np.float32)

    xb = x.astype(NPBF)
    xdb = xdot.astype(NPBF)
    hb = h.astype(NPBF)

    in_maps = []
    for c in range(N_CORES):
        sl = slice(c * BS, (c + 1) * BS)
        xt = xb[sl].T                                         # [64, BS]
        xdt = xdb[sl].T
        ht = hb[sl].T                                         # [128, BS]
        inb = np.empty((NPAIR, 128, PACKW), NPBF)
        for p in range(NPAIR):
            cls = slice(p * PAIR, (p + 1) * PAIR)
            inb[p, 0:64, 0:PAIR] = xt[:, cls]
            inb[p, 64:128, 0:PAIR] = xdt[:, cls]
            inb[p, :, PAIR:PACKW] = ht[:, cls]
        in_maps.append({"inb": inb, "wpack": wpack, "bpack": bpack})
    return in_maps


def kernel(**inputs) -> np.ndarray:
    in_maps = _prep_in_maps(**inputs)
    nc = _get_nc()
    res = bass_utils.run_bass_kernel_spmd(nc, in_maps,
                                          core_ids=list(range(N_CORES)))
    out = np.empty((B, H), np.float32)
    for c in range(N_CORES):
        out[c * BS:(c + 1) * BS] = res.results[c]["outt"].T.astype(np.float32)
    return out


# revision 23
# speedup vs baseline: 1.0627x; 1.0172x over previous
"""Trainium2 Bass kernel for the Neural-CDE-style cell (nn_JaCDE_88167088653055).

Math (per batch row b):
    x    = spline(coeffs, t)   xdot = spline(dcoeffs, t)
    l1   = x @ wx.T + h @ wh.T + b0
    relu = relu(l1);  dr = sigmoid(l1)
    lout = relu @ wout.T + b1; dth = 1 - tanh(lout)^2
    m1   = (dr*(xdot @ wx.T)) @ wout.T
    jx   = dth*m1;  m2 = (dr*(jx @ wh.T)) @ wout.T
    jxh  = dth*m2;  m3 = (dr*(jxh @ wh.T)) @ wout.T
    out  = dth*(m1 + m2 + m3)

Device-side reformulation (bf16 PE path; tolerance 2e-2):
  * host does the spline; x/xdot stack on partitions 0:64 / 64:128.
  * single ACT table set: walrus maps funcs first-fit to table sets
    (Relu/Tanh/Square -> exp_and_others, Sigmoid -> sigmoid_and_others), so
    sigmoid is computed as dr = (1+tanh(l1/2))/2: every dr-multiply is an
    STT (tt+1)*g = 2*dr*g and the extra 2 folds into the wout copies
    (won2 = -wout.T/2).  dth-multiplies are STT (sq-1)*(-m) with
    sq = Square(tanh(lout)).  -> ONE ACT_TABLE_LOAD, during the DMA wait.
  * relu runs on the (otherwise idle pre-chain) DVE as STT (l1+b0) max 0,
    cutting the serial ACT queue from 8 ops to 6.
  * m1+m2+m3 accumulates IN PSUM: A = m1; (read jx); A += m2; (read
    jx+jxh -> feeds g1+g2 so the next matmul produces m2+m3); A += wop*p2
    (subtracts the extra m2) and A += won*p23. No vector adds at all.
  * all matmuls are single N=512; batch = 2 chains of 512 per core.
  * small HAM warm-up trickle fills the input-DMA dead window.
  * one input DMA per pair (pair0 sync ring, pair1 gpsimd SWDGE);
    weights on the scalar ring; outputs DMA per pair as soon as ready.

Sharding: pure data parallel - batch 8192 = 8 cores x 1024 rows; weights
replicated. Activations feature-major ([H or CIN on partitions, batch free]);
every matmul is out.T = W @ act.T with contraction on partitions.
"""

import ml_dtypes
import numpy as np

import concourse.bass as bass
import concourse.mybir as mybir
import concourse.tile as tile
from concourse import bacc, bass_utils

N_CORES = 8
B = 8192
NOBS = 16
CIN = 64
H = 128
BS = B // N_CORES       # 1024 batch rows per core
PAIR = 512              # batch columns per chain (= one PSUM bank)
NPAIR = BS // PAIR      # 2 chains per core
PACKW = 2 * PAIR        # input pack: [x|xd (512) | h (512)]
F32 = mybir.dt.float32
BF16 = mybir.dt.bfloat16
NPBF = ml_dtypes.bfloat16

N_WARM = 16             # HAM warm-up trickle fills the input-DMA dead window

_NC_CACHE = {}


def _build_nc():
    AF = mybir.ActivationFunctionType
    OP = mybir.AluOpType

    nc = bacc.Bacc("TRN2", target_bir_lowering=False, debug=False,
                   enable_asserts=False, num_devices=N_CORES)

    # per-pair input pack: cols 0:512 = x.T (parts 0:64) | xdot.T (64:128),
    # cols 512:1024 = h.T
    inb = nc.dram_tensor("inb", [NPAIR, 128, PACKW], BF16,
                         kind="ExternalInput")
    # [wxx | whT | won2 | wop2 | woT]; wxx rows 0:64 = wx.T (x), 64:128 =
    # wx.T (xdot); won2 = -0.5*wout.T, wop2 = +0.5*wout.T, woT = wout.T
    wpack = nc.dram_tensor("wpack", [128, 5 * H], BF16, kind="ExternalInput")
    bpack = nc.dram_tensor("bpack", [128, 4], F32, kind="ExternalInput")
    outt = nc.dram_tensor("outt", [H, BS], BF16, kind="ExternalOutput")

    def mm(out_ap, lhsT, rhs, start=True, stop=True):
        nc.tensor.matmul(out_ap, lhsT, rhs, start=start, stop=stop,
                         skip_group_check=True)

    with tile.TileContext(nc) as tc:
        with tc.tile_pool(name="w", bufs=1) as wp, \
             tc.tile_pool(name="io", bufs=2) as io, \
             tc.tile_pool(name="tmp", bufs=2) as tmp, \
             tc.tile_pool(name="fr", bufs=4, space="PSUM") as fr, \
             tc.tile_pool(name="acc", bufs=2, space="PSUM") as accp, \
             tc.tile_pool(name="g", bufs=2, space="PSUM") as gp:

            # --- weights / biases on the scalar (ACT) HWDGE ring ----------
            ws = wp.tile([128, 5 * H], BF16, tag="ws")
            nc.scalar.dma_start(ws[:], wpack[:])
            bs_ = wp.tile([128, 4], F32, tag="bs")
            nc.scalar.dma_start(bs_[:], bpack[:])
            wxx_x = ws[0:64, 0:H]
            wxx_xd = ws[64:128, 0:H]
            whT = ws[:, H:2 * H]
            won2 = ws[:, 2 * H:3 * H]
            wop2 = ws[:, 3 * H:4 * H]
            woT = ws[:, 4 * H:5 * H]
            b0s = bs_[:, 0:1]
            b0h = bs_[:, 1:2]    # = b0/2
            b1s = bs_[:, 2:3]    # = b1
            mone = bs_[:, 3:4]   # = -1.0

            # --- input DMAs: one per pair, separate rings -----------------
            it = [io.tile([128, PACKW], BF16, tag="it", name=f"it{p}")
                  for p in range(NPAIR)]
            nc.sync.dma_start(it[0][:], inb[0])
            nc.gpsimd.dma_start(it[1][:], inb[1])

            # memset scratch (DVE is idle): trickle weights + relu zeros
            wdum = wp.tile([128, 128], BF16, tag="wdum")
            nc.vector.memset(wdum[:], 0.0)
            zs = wp.tile([128, PAIR], BF16, tag="zs")
            nc.vector.memset(zs[:], 0.0)
            # --- HAM warm-up trickle (PE queue head) ----------------------
            dumg = gp.tile([H, PAIR], F32, tag="g")
            for _ in range(N_WARM):
                mm(dumg[:, 0:128], wdum[:], wdum[:], start=True, stop=True)

            T = {}

            def tt(name, p, dtype=BF16):
                t = tmp.tile([H, PAIR], dtype, tag=name, name=f"{name}{p}")
                T[(name, p)] = t
                return t

            pairs = range(NPAIR)

            # --- front matmuls --------------------------------------------
            def front_mms(p):
                L = fr.tile([H, PAIR], F32, tag="fr", name=f"L{p}")
                T[("L", p)] = L
                mm(L[:], wxx_x, it[p][0:64, 0:PAIR], start=True, stop=False)
                mm(L[:], whT, it[p][:, PAIR:PACKW], start=False, stop=True)
                # U shares the acc pool: A[p] reuses U[p]'s bank, and the
                # WAR (A's first matmul after p1's read of U) is a true
                # data dependency anyway.  U (K=64, row group 2/3) runs
                # concurrently with a K=64 row-group-0/1 matmul.
                U = accp.tile([H, PAIR], F32, tag="acc", name=f"U{p}")
                T[("U", p)] = U
                mm(U[:], wxx_xd, it[p][64:128, 0:PAIR], start=True, stop=True)

            # --- relu on DVE: (l1 + b0) max 0 -----------------------------
            def relu(p):
                nc.vector.scalar_tensor_tensor(
                    tt("relu", p)[:], T[("L", p)][:], b0s, zs[:],
                    OP.add, OP.max)

            # --- ACT front: tt/th/sq only (all exp_and_others) ------------
            def act_tt(p):
                nc.scalar.activation(tt("tt", p)[:], T[("L", p)][:],
                                     AF.Tanh, bias=b0h, scale=0.5)

            def lo_mm(p):
                LO = fr.tile([H, PAIR], F32, tag="fr", name=f"LO{p}")
                T[("LO", p)] = LO
                mm(LO[:], woT, T[("relu", p)][:], start=True, stop=True)

            def act_th_sq(p):
                nc.scalar.activation(tt("th", p)[:], T[("LO", p)][:],
                                     AF.Tanh, bias=b1s)
                nc.scalar.activation(tt("sq", p)[:], T[("th", p)][:],
                                     AF.Square, bias=0.0)

            A = {}

            def p1(p):
                # (tt+1)*u = 2*dr*u; the 2 folds into won2/wop2
                nc.vector.scalar_tensor_tensor(
                    tt("p1", p)[:], T[("tt", p)][:], 1.0, T[("U", p)][:],
                    OP.add, OP.mult)

            def y(name, p):
                # (sq-1)*A = (1-tanh^2)*m with A = -(m1+...)
                nc.vector.scalar_tensor_tensor(
                    tt(name, p)[:], T[("sq", p)][:], 1.0, A[p][:],
                    OP.subtract, OP.mult)

            def pg(name, p, gt):
                nc.vector.scalar_tensor_tensor(
                    tt(name, p)[:], T[("tt", p)][:], 1.0, gt[:],
                    OP.add, OP.mult)

            # front, in expected-readiness order; pair0's LO matmul sits
            # ahead of pair1's front matmuls in the PE FIFO so pair0's
            # th/sq path is not blocked behind them.
            front_mms(0)
            front_mms(1)
            relu(0)                                   # DVE
            act_tt(0)                                 # ACT
            lo_mm(0)                                  # PE
            act_tt(1)                                 # ACT
            relu(1)                                   # DVE
            act_th_sq(0)                              # ACT
            lo_mm(1)                                  # PE
            act_th_sq(1)                              # ACT

            for p in pairs:
                A[p] = accp.tile([H, PAIR], F32, tag="acc", name=f"A{p}")
            G = {}

            def g_mm(name, p, src):
                G[(name, p)] = gp.tile([H, PAIR], F32, tag="g",
                                       name=f"{name}_{p}")
                mm(G[(name, p)][:], whT, src[:], start=True, stop=True)

            # --- Jacobian chains (A accumulates -(m1+m2+m3)) --------------
            p1(0)                                                  # DVE
            mm(A[0][:], won2, T[("p1", 0)][:], start=True, stop=False)
            y("y1", 0)                                             # DVE
            g_mm("G1", 0, T[("y1", 0)])
            p1(1)                                                  # DVE
            mm(A[1][:], won2, T[("p1", 1)][:], start=True, stop=False)
            pg("p2", 0, G[("G1", 0)])                              # DVE
            mm(A[0][:], won2, T[("p2", 0)][:], start=False, stop=False)
            y("y1", 1)                                             # DVE
            g_mm("G1", 1, T[("y1", 1)])
            y("y2", 0)                                             # DVE
            g_mm("G2", 0, T[("y2", 0)])
            mm(A[0][:], wop2, T[("p2", 0)][:], start=False, stop=False)
            pg("p2", 1, G[("G1", 1)])                              # DVE
            mm(A[1][:], won2, T[("p2", 1)][:], start=False, stop=False)
            pg("p23", 0, G[("G2", 0)])                             # DVE
            mm(A[0][:], won2, T[("p23", 0)][:], start=False, stop=True)
            y("y2", 1)                                             # DVE
            g_mm("G2", 1, T[("y2", 1)])
            mm(A[1][:], wop2, T[("p2", 1)][:], start=False, stop=False)
            y("out", 0)                                            # DVE
            nc.sync.dma_start(outt[:, 0:PAIR], T[("out", 0)][:])
            pg("p23", 1, G[("G2", 1)])                             # DVE
            mm(A[1][:], won2, T[("p23", 1)][:], start=False, stop=True)
            y("out", 1)                                            # DVE
            nc.scalar.dma_start(outt[:, PAIR:2 * PAIR], T[("out", 1)][:])

    nc.compile()
    return nc


def _get_nc():
    if "nc" not in _NC_CACHE:
        _NC_CACHE["nc"] = _build_nc()
    return _NC_CACHE["nc"]


def _prep_in_maps(t, h, coeffs, dcoeffs, tobs, wx, wh, wout, b0, b1):
    t = np.asarray(t, np.float32)
    h = np.asarray(h, np.float32)
    coeffs = np.asarray(coeffs, np.float32)
    dcoeffs = np.asarray(dcoeffs, np.float32)
    tobs = np.asarray(tobs, np.float32)
    wx = np.asarray(wx, np.float32)
    wh = np.asarray(wh, np.float32)
    wout = np.asarray(wout, np.float32)
    b0 = np.asarray(b0, np.float32)
    b1 = np.asarray(b1, np.float32)

    ts = t[0]
    idx = int(np.clip(np.searchsorted(tobs, ts, side="right") - 1, 0, NOBS - 2))
    dtv = np.float32(ts - tobs[idx])
    powers = dtv ** np.arange(4, dtype=np.float32)            # [4]

    # host-side spline eval
    x = coeffs[:, idx] @ powers                               # [B, CIN]
    xdot = dcoeffs[:, idx] @ powers                           # [B, CIN]

    wxx = np.concatenate([wx.T, wx.T], axis=0)                # [128, 128]
    wpack = np.concatenate(
        [wxx, wh.T, -0.5 * wout.T, 0.5 * wout.T, wout.T],
        axis=1).astype(NPBF)                                  # [128, 640]
    bpack = np.stack([b0, 0.5 * b0, b1, np.full(H, -1.0, np.float32)],
                     axis=1).astype(np.float32)

    xb = x.astype(NPBF)
    xdb = xdot.astype(NPBF)
    hb = h.astype(NPBF)

    in_maps = []
    for c in range(N_CORES):
        sl = slice(c * BS, (c + 1) * BS)
        xt = xb[sl].T                                         # [64, BS]
        xdt = xdb[sl].T
        ht = hb[sl].T                                         # [128, BS]
        inb = np.empty((NPAIR, 128, PACKW), NPBF)
        for p in range(NPAIR):
            cls = slice(p * PAIR, (p + 1) * PAIR)
            inb[p, 0:64, 0:PAIR] = xt[:, cls]
            inb[p, 64:128, 0:PAIR] = xdt[:, cls]
            inb[p, :, PAIR:PACKW] = ht[:, cls]
        in_maps.append({"inb": inb, "wpack": wpack, "bpack": bpack})
    return in_maps


def kernel(**inputs) -> np.ndarray:
    in_maps = _prep_in_maps(**inputs)
    nc = _get_nc()
    res = bass_utils.run_bass_kernel_spmd(nc, in_maps,
                                          core_ids=list(range(N_CORES)))
    out = np.empty((B, H), np.float32)
    for c in range(N_CORES):
        out[c * BS:(c + 1) * BS] = res.results[c]["outt"].T.astype(np.float32)
    return out
